# revision 35
# baseline (speedup 1.0000x reference)
"""AffinityLoss Trainium2 kernel (fp8 DoubleRow edition).

loss = mean_b( ||x_b x_b^T||_F^2 + ||y_b y_b^T||_F^2 - 2 ||x_b y_b^T||_F^2 )

with x_b (20, N), y_b (4, N), N = 257*400 = 102800.

Strategy: stack z = [x; y] (24, N) per batch.  With sign vector
sigma = (+1)*20 ++ (-1)*4 and G = z z^T (24, 24):

    loss_b = sum_{d,e} sigma_d sigma_e G[d,e]^2

Data-parallel over batch: 2 batches per core on 8 cores.

The tensor engine contracts over the partition axis, so the host pre-folds
z into a partition-major chunk layout and casts fp32 -> fp8e4 (e4m3).  The
quantization bias on the loss is ~1.4e-3 relative — far inside the 2e-2
gate — while halving HBM traffic vs bf16 and, via the fp8 DoubleRow perf
mode, letting each matmul contract TWO 128-deep n-chunks per PE pass.

Per batch, n splits exactly into 402 pair-groups of 2 chunks (KPACK=1: no
column packing, zero chunk padding since 804 = 402*2).  One DoubleRow
matmul per pair-group accumulates directly into a minimal (24, 24) PSUM
Gram tile at 5 ns/matmul — the same 2.5 ns/chunk PE rate as wider
KPACK packings (PE cost scales with output free size per chunk), but the
PSUM->SBUF evacuation shrinks to a 150 ns [24,24] DVE copy.
Hardware restriction (walrus `s3_lw_dual_fp8_restrictions`, isolated by
probing: 48/96/144+ pass, 120/360 fail): the dual-fp8 LDWEIGHTS k-tile
stride must be a multiple of 16 bytes.  Each SBUF tile is laid out
k-tile-plane-major ([128, 2, tg, 24], pair stride tg*24), so tile
group-counts tg are kept even.  The host fold emits columns per tile as
[plane 0 | plane 1].

Loads are spread over THREE DMA queues (SP + Activation HWDGE rings and
the Pool-engine SWDGE ring) in equal waves so tiles complete in PE
consumption order; after the first wave lands the PE runs back-to-back
5 ns DoubleRow matmuls and is the critical engine.  Each batch's Gram
goes into one fused (24, 48) staging tile, written out by a single DMA;
the host sums the two diagonal 24x24 blocks per batch and does the tiny
signed square-sum + mean in f64.

CoreSim cost-model timeline (per core): the split XBAR-transposed
first wave anchors at 1731 ns (one 14 ns 16-row XBAR slice + 1717 ns DGE
init; riders complete by the anchor), the PE then runs 804 back-to-back
matmuls (mid p-state until the 3 us mark), and the tail (100 ns sem +
150 ns PSUM copy + 100 ns sem + 500 ns out-DMA descriptor floor +
1717 ns out-DMA DGE init) is pure fixed latency -> 8953 ns total, vs
22034 ns for the bf16 2-queue tile-framework baseline.  Every component
sits at a probed cost-model floor.
"""

import os
import sys

import numpy as np

_TRN_REPO = "/opt/trn_rl_repo"
if os.path.isdir(_TRN_REPO) and _TRN_REPO not in sys.path:
    sys.path.insert(0, _TRN_REPO)

B, D, S, H, W = 16, 20, 4, 257, 400
N = H * W                  # 102800
R = D + S                  # 24 z-rows
NCORES = 8
BPC = B // NCORES          # 2 batches per core
KPACK = 1                  # column blocks per matmul (1*24 = 24 out rows)
PAIR = 2                   # k-tiles per matmul (fp8 DoubleRow)
PPART = KPACK * R          # 24
GCOLS = PAIR * PPART       # 48 tile columns per group
CPG = KPACK * PAIR         # 2 n-chunks per group
GROUPS = -(-(-(-N // 128)) // CPG)  # ceil(804/2) = 402, exact: no padding
NPAD = GROUPS * CPG * 128          # 102912

# Per-batch DMA tiling (in groups) and queue per load, round-robin over
# (SP, ACT, POOL).  Equal waves across the three queues keep tiles
# completing in PE order; the first 26-group wave is the most that fits
# under the 500 ns descriptor-generation floor of the first DMA.
TILES = (
    (2, 2, 24, 18, 24, 28, 56, 56, 56, 48, 46, 42),
    (52, 52, 52, 56, 56, 56, 26, 26, 26),
)
NXP = 2        # first two batch-0 loads (SP/ACT rings) use the XBAR
XPW = 2 * R * PAIR // 2  # 48 u16 rows per transposed tile
QUEUES = ("sync", "scalar", "gpsimd")

_nc_cache = None


def _build():
    """Raw-Bass build (no TileContext): hand-rolled semaphores avoid the
    tile framework's end-of-kernel drain chain, and the all-engine
    entry/exit barriers are elided (~600 ns combined).  Safe because every
    cross-engine dependency is explicitly semaphore-guarded: PE waits on
    each load's DMA sem, DVE on PE's accumulator sems, the out DMA on the
    copy sem, and SP's final wait on the out DMA sem transitively orders
    every DMA before the last engine halts.  The const-AP memsets the
    barrier normally orders have no consumers in this kernel."""
    global _nc_cache
    if _nc_cache is not None:
        return _nc_cache

    import concourse.bass as bass
    import concourse.mybir as mybir

    f32 = mybir.dt.float32
    fp8 = mybir.dt.float8e4
    perf = mybir.MatmulPerfMode.DoubleRow

    _orig_barrier = bass.Bass.all_engine_barrier
    bass.Bass.all_engine_barrier = lambda self, *a, **k: None
    try:
        nc = _build_inner(bass, mybir, f32, fp8, perf)
    finally:
        bass.Bass.all_engine_barrier = _orig_barrier
    _nc_cache = nc
    return nc


def _build_inner(bass, mybir, f32, fp8, perf):
    u16 = mybir.dt.uint16
    nc = bass.Bass(target_bir_lowering=False)
    z_t = nc.dram_tensor("z", (BPC, 128, GROUPS * GCOLS), fp8,
                         kind="ExternalInput")
    zx_t = nc.dram_tensor("zx", (NXP, XPW, 128), u16, kind="ExternalInput")
    out_t = nc.dram_tensor("out", (PPART, BPC * PPART), f32,
                           kind="ExternalOutput")

    # flat load list: (batch, load_idx, tg, col0, queue)
    loads = []
    qi = 0
    for b in range(BPC):
        g0 = 0
        for tg in TILES[b]:
            loads.append((b, len(loads), tg, g0 * GCOLS, QUEUES[qi % 3]))
            qi += 1
            g0 += tg

    with bass.ExitStack() as ctx:
        zf = [
            ctx.enter_context(
                nc.sbuf_tensor(f"zf{b}", [128, GROUPS * GCOLS], fp8))
            for b in range(BPC)
        ]
        gsb = ctx.enter_context(
            nc.sbuf_tensor("gsb", [PPART, BPC * PPART], f32))
        acc = [
            ctx.enter_context(nc.psum_tensor(f"acc{b}", [PPART, PPART], f32))
            for b in range(BPC)
        ]
        dsem = [
            ctx.enter_context(nc.semaphore(f"ds{j}")) for j in range(len(loads))
        ]
        psem = [ctx.enter_context(nc.semaphore(f"ps{b}")) for b in range(BPC)]
        csem = ctx.enter_context(nc.semaphore("cs"))
        osem = ctx.enter_context(nc.semaphore("os"))

        # The zero-padding makes each batch's final group (chunks >= 810)
        # all-zero, so its matmul contributes nothing: skip it.
        skip_last_mm = GROUPS * CPG - (-(-N // 128)) >= CPG

        with nc.Block() as block:

            def dma_prog(eng, qname):
                for b, j, tg, c0, q in loads:
                    if q != qname:
                        continue
                    if b == 0 and j < NXP:
                        # XBAR-transposed first wave (SP/ACT only): no
                        # 500 ns descriptor floor, split into three 16-row
                        # slices so the ring anchors its first completion
                        # at 14 + 1717 = 1731 ns (the two rider slices
                        # complete by the anchor); the PE starts ~490 ns
                        # sooner than a regular 500 ns-floor first DMA.
                        dst = zf[b][:, c0:c0 + tg * GCOLS].bitcast(
                            mybir.dt.uint16)
                        for s in range(3):
                            eng.dma_start_transpose(
                                dst[:, 16 * s:16 * (s + 1)],
                                zx_t[j][16 * s:16 * (s + 1), :],
                            ).then_inc(dsem[j], 16)
                    else:
                        eng.dma_start(
                            zf[b][:, c0:c0 + tg * GCOLS],
                            z_t[b][:, c0:c0 + tg * GCOLS],
                        ).then_inc(dsem[j], 16)
                if qname == "sync":
                    eng.wait_ge(csem, BPC)
                    eng.dma_start(out_t[:, :], gsb[:, :]).then_inc(osem, 16)
                    eng.wait_ge(osem, 16)

            @block.sync
            def _(sync):
                dma_prog(sync, "sync")

            @block.scalar
            def _(scalar):
                dma_prog(scalar, "scalar")

            @block.gpsimd
            def _(gpsimd):
                dma_prog(gpsimd, "gpsimd")

            @block.tensor
            def _(tensor):
                for b, j, tg, c0, q in loads:
                    tensor.wait_ge(dsem[j], 48 if (b == 0 and j < NXP) else 16)
                    first_tile = (j == 0) or (loads[j - 1][0] != b)
                    last_tile = (j == len(loads) - 1) or (loads[j + 1][0] != b)
                    planes = zf[b][:, c0:c0 + tg * GCOLS].rearrange(
                        "p (i c) -> p i c", i=PAIR)
                    g_hi = tg - 1 if (last_tile and skip_last_mm) else tg
                    for g in range(g_hi):
                        sl = planes[:, :, g * PPART:(g + 1) * PPART]
                        mm = tensor.matmul(
                            acc[b][:, :], sl, sl,
                            start=(first_tile and g == 0),
                            stop=(last_tile and g == g_hi - 1),
                            perf_mode=perf,
                        )
                        if last_tile and g == g_hi - 1:
                            mm.then_inc(psem[b], 1)

            @block.vector
            def _(vector):
                for b in range(BPC):
                    # PSUM->SBUF evacuation (DVE; GPSIMD cannot access PSUM)
                    vector.wait_ge(psem[b], 1)
                    vector.tensor_copy(
                        gsb[:, b * PPART:(b + 1) * PPART], acc[b][:, :]
                    ).then_inc(csem, 1)

    nc.finalize()
    return nc


def _fold(z):
    """z: (nb, R, N) float32 -> (nb, 128, GROUPS*GCOLS) fp8 folded layout.

    Chunk c = (g*2 + i)*KPACK + k feeds column block (i, g, k); within each
    DMA tile of tg groups the column order is plane-major:
    [i=0: g_local x (k, r) | i=1: g_local x (k, r)] to satisfy the dual-fp8
    LDWEIGHTS k-tile stride >= 128 rule.
    """
    import ml_dtypes

    nb = z.shape[0]
    zp = np.zeros((nb, R, NPAD), dtype=ml_dtypes.float8_e4m3)
    zp[:, :, :N] = z.astype(ml_dtypes.float8_e4m3)
    # (nb, p, g, i, k, r): chunk (g*2+i)*KPACK+k at partition p, row r
    zf = zp.reshape(nb, R, GROUPS, PAIR, KPACK, 128).transpose(0, 5, 2, 3, 4, 1)
    out = np.empty((nb, 128, GROUPS * GCOLS), dtype=ml_dtypes.float8_e4m3)
    for b in range(nb):
        cols = []
        g0 = 0
        for tg in TILES[b % BPC]:
            blk = zf[b][:, g0:g0 + tg]        # (p, tg, 2, k, r)
            blk = blk.transpose(0, 2, 1, 3, 4)  # (p, 2, tg, k, r)
            cols.append(blk.reshape(128, tg * GCOLS))
            g0 += tg
        assert g0 == GROUPS
        out[b] = np.concatenate(cols, axis=1)
    return out


def _fold_xp(zc):
    """Transposed u16 view of batch-0's first NXP*XPW*2 fp8 columns."""
    zx = np.empty((NXP, XPW, 128), dtype=np.uint16)
    for j in range(NXP):
        blk = zc[0][:, 2 * XPW * j:2 * XPW * (j + 1)]   # (128, 2*XPW) fp8
        u = blk.view(np.uint8).reshape(128, XPW, 2)
        zx[j] = (u[:, :, 0].astype(np.uint16)
                 | (u[:, :, 1].astype(np.uint16) << 8)).T
    return zx


def _make_in_maps(input, target):
    input = np.asarray(input, dtype=np.float32).reshape(B, D, N)
    target = np.asarray(target, dtype=np.float32).reshape(B, S, N)
    z = np.concatenate([input, target], axis=1)
    zf = _fold(z)
    maps = []
    for c in range(NCORES):
        zc = np.ascontiguousarray(zf[c * BPC:(c + 1) * BPC])
        maps.append({"z": zc, "zx": _fold_xp(zc)})
    return maps


def _host_reduce(results):
    total = np.float64(0.0)
    for r in results:
        out = np.asarray(r["out"], dtype=np.float64)  # (120, BPC*120)
        for b in range(BPC):
            blocks = out[:, b * PPART:(b + 1) * PPART]
            blocks = blocks.reshape(KPACK, R, KPACK, R)
            G = sum(blocks[i, :, i, :] for i in range(KPACK))  # (24, 24)
            total += np.sum(G * G) - 4.0 * np.sum(G[:D, D:] ** 2)
    total /= B
    return np.asarray(total, dtype=np.float32).reshape(())


def run(input, target, trace=False, **kwargs):
    """Run the SPMD kernel on cores 0..7; returns (loss, BassKernelResults)."""
    import time

    from concourse.bass_utils import run_bass_kernel_spmd

    nc = _build()
    in_maps = _make_in_maps(input, target)
    try:
        res = run_bass_kernel_spmd(
            nc, in_maps, core_ids=list(range(NCORES)), trace=trace, **kwargs
        )
    except Exception:
        # transient accelerator states (e.g. a prior crashed process) have
        # been observed to clear after ~30s; retry once
        time.sleep(30)
        res = run_bass_kernel_spmd(
            nc, in_maps, core_ids=list(range(NCORES)), trace=trace, **kwargs
        )
    return _host_reduce(res.results), res


def kernel(input, target):
    loss, _ = run(input, target, trace=False)
    return loss


if __name__ == "__main__":
    rng = np.random.default_rng(0)
    inp = rng.standard_normal((B, D, H, W), dtype=np.float32)
    tgt = rng.standard_normal((B, S, H, W), dtype=np.float32)
    got = kernel(input=inp, target=tgt)
    x = inp.reshape(B, D, -1).astype(np.float64)
    y = tgt.reshape(B, S, -1).astype(np.float64)
    gxx = np.einsum("bdn,ben->bde", x, x)
    gyy = np.einsum("bsn,btn->bst", y, y)
    gxy = np.einsum("bdn,bsn->bds", x, y)
    want = np.mean(
        (gxx ** 2).sum((1, 2)) + (gyy ** 2).sum((1, 2)) - 2 * (gxy ** 2).sum((1, 2))
    )
    print("got", got, "want", want, "rel", abs(got - want) / abs(want))
